# revision 1
# baseline (speedup 1.0000x reference)
import sys

for _p in ("/opt/trn_rl_repo", "/opt/trn_rl_repo/concourse"):
    if _p not in sys.path:
        sys.path.insert(0, _p)

import numpy as np
import ml_dtypes

N_CORES = 8
B, H, W_DIM, C = 8, 32, 32, 288
N = H * W_DIM          # 1024 points per core (batch-dim sharding: 1 image per core)
O = 64                 # codewords
CP = 384               # C padded to 3 full 128-partition tiles
NCT = CP // 128        # 3 c-tiles
CHUNK = 512            # PSUM bank free size (fp32)
NCH = N // CHUNK       # 2 chunks

_CACHE = {}


def _patch_drain_split():
    # The end-of-TileContext drain waits on the FULL global clock (PE + DVE
    # + one sem per DMA HW queue = 10 components here), overflowing the
    # CTRL_NO struct's sync-wait slots in walrus. Split: emit one 1-wait SP
    # nop per clock component first; the original drain's full-clock
    # add_sem_waits then elides everything via SP wait history.
    import concourse.tile as tile_mod
    from concourse.vector_clock import ScopedClock, VectorClock

    if getattr(tile_mod.TileContext, "_drain_split_patched", False):
        return

    def _drain_and_barrier(self, tick_clock, wait_clock):
        gc = tick_clock.global_clock
        for idx in range(len(gc)):
            tick = gc[idx]
            if tick <= 0:
                continue
            nop = self.nc.sync.nop(nofuse=True, hint="drain_split")
            vc = VectorClock()
            vc.require_at_least(idx, tick)
            wait_clock.add_sem_waits(nop.ins, ScopedClock({None: vc}))
        # Waitless drain: the nops above (same SP sequencer, in order)
        # already guarantee every sem is at its final value here.
        self.nc.sync.drain()
        self.nc.all_engine_barrier()
        assert self.sems is not None
        popped = self.nc._tile_sem_poison_stack.pop()
        assert popped is self._sem_poison
        self.nc.clear_and_free_semaphores(list(self.sems.allocated().values()))
        self.nc.all_engine_barrier()

    tile_mod.TileContext._drain_and_barrier = _drain_and_barrier
    tile_mod.TileContext._drain_split_patched = True


def _build_program():
    import concourse.bass as bass
    import concourse.tile as tile
    from concourse import mybir

    _patch_drain_split()
    nc = bass.Bass("TRN2", debug=False, num_devices=N_CORES)

    xt_d = nc.dram_tensor("xt", [CP, N], mybir.dt.bfloat16, kind="ExternalInput")
    w_d = nc.dram_tensor("wp", [CP, O], mybir.dt.float32, kind="ExternalInput")
    b_d = nc.dram_tensor("bvec", [O, 1], mybir.dt.float32, kind="ExternalInput")
    out_d = nc.dram_tensor("out_t", [O, N], mybir.dt.float32, kind="ExternalOutput")

    xt = xt_d.ap()
    wp = w_d.ap()
    bvec = b_d.ap()
    out_t = out_d.ap()

    from contextlib import ExitStack

    with tile.TileContext(nc) as tc, ExitStack() as ctx:
        const_pool = ctx.enter_context(tc.tile_pool(name="const", bufs=1))
        abs_pool = ctx.enter_context(tc.tile_pool(name="abs", bufs=2))
        psum_pool = ctx.enter_context(tc.tile_pool(name="ps", bufs=1, space="PSUM"))

        # Walrus TensorScalar ISA structs fit only ONE sync wait, and tile's
        # wait assigner emits same-engine sem waits too (optimize_sems is
        # disabled). Discipline: every instruction must need <=1 fresh wait.
        # Tiny DVE "touch" ops absorb extra waits; each touch writes its own
        # scratch column so touches never WAW-chain each other.
        scratch = const_pool.tile([1, 512], mybir.dt.float32)
        touch_col = [0]

        def touch(src_ap):
            k = touch_col[0]
            touch_col[0] += 1
            nc.vector.tensor_scalar_add(
                scratch[0:1, k : k + 1], src_ap, 0.0
            )
            return k

        # A PSUM bank's region clock joins the DVE incs of every reader
        # killed by a later write, so any bank read after a touch inherits
        # that touch's inc as a 2nd wait. The absorber (a scratch-to-scratch
        # touch RAW on the PSUM touch) pre-loads that inc into DVE wait
        # history so the inherited component is elided everywhere after.
        def ptouch(bank_tile):
            k = touch(bank_tile[0:1, 0:1])
            touch(scratch[0:1, k : k + 1])

        x_sb = []
        w_sb = []
        for ct in range(NCT):
            xti = const_pool.tile([128, N], mybir.dt.bfloat16, name=f"x_sb{ct}")
            nc.sync.dma_start(xti[:], xt[128 * ct : 128 * (ct + 1), :])
            touch(xti[0:1, 0:1])
            x_sb.append(xti)
            wti = const_pool.tile([128, O], mybir.dt.float32, name=f"w_sb{ct}")
            nc.sync.dma_start(wti[:], wp[128 * ct : 128 * (ct + 1), :])
            touch(wti[0:1, 0:1])
            w_sb.append(wti)

        b_sb = const_pool.tile([O, 1], mybir.dt.float32)
        nc.sync.dma_start(b_sb[:], bvec[:, :])
        touch(b_sb[0:1, 0:1])

        # Z[:, 64] = 1, else 0. lhsT for codeword o = Z[:, 64-o : 192-o],
        # a one-hot-column matrix routing Σ_c(rhs) onto PSUM partition o.
        zwin = const_pool.tile([128, O + 128], mybir.dt.bfloat16)
        nc.vector.memset(zwin[:], 0.0)
        nc.vector.memset(zwin[:, O : O + 1], 1.0)

        out_sb = const_pool.tile([O, N], mybir.dt.float32)

        # Parity PSUM banks: pb[o%2][ch]. Odd/even o accumulate separately so
        # the per-block PSUM touch's WAR lands on block o+2's matmul (a full
        # o-block of slack), never stalling the PE pipeline.
        pb = [
            [
                psum_pool.tile([128, CHUNK], mybir.dt.float32, name=f"pb{p}{ch}")
                for ch in range(NCH)
            ]
            for p in range(2)
        ]
        # Touch-target bank: tiny start+stop singleton matmuls land here, so
        # touches read a closed group (never mid-accumulation) whose PE inc
        # dominates the preceding real matmul's.
        tinyb = psum_pool.tile([128, CHUNK], mybir.dt.float32, name="tinyb")

        for o in range(O):
            p = o % 2
            abs_ts = []
            for ct in range(NCT):
                a = abs_pool.tile(
                    [128, N], mybir.dt.bfloat16, name=f"abs{ct}", tag=f"abs{ct}"
                )
                abs_ts.append(a)
                # abs_max is not encodable (walrus is_valid_aluop); 2-op
                # |x-w| = max(-(x-w), x-w). TS t-write: DVE self WAR on prior
                # STT's read, elided after the first pair via STT RAW history.
                t = abs_pool.tile(
                    [128, N], mybir.dt.bfloat16, name=f"t{ct}", tag=f"t{ct}"
                )
                nc.vector.tensor_scalar_sub(
                    t[:], x_sb[ct][:], w_sb[ct][:, o : o + 1]
                )
                nc.vector.scalar_tensor_tensor(
                    a[:], t[:], -1.0, t[:],
                    op0=mybir.AluOpType.mult,
                    op1=mybir.AluOpType.max,
                )
            for ch in range(NCH):
                for ct in range(NCT):
                    nc.tensor.matmul(
                        pb[p][ch][:],
                        lhsT=zwin[:, O - o : O - o + 128],
                        rhs=abs_ts[ct][:, CHUNK * ch : CHUNK * (ch + 1)],
                        start=(o == p and ct == 0),
                        stop=(o == O - 2 + p and ct == NCT - 1),
                    )
                    if ch == 1 and o <= O - 3:
                        # Tiny singleton matmul + touch: ready at the same
                        # scheduling event as abs(o+2, ct) but wins on
                        # priority, loading a PE sem value covering this
                        # matmul into DVE wait history so abs(o+2, ct)'s
                        # PE WAR wait is elided.
                        nc.tensor.matmul(
                            tinyb[0:1, 0:1],
                            lhsT=zwin[:, O : O + 1],
                            rhs=abs_ts[ct][:, 0:1],
                            start=True,
                            stop=True,
                        )
                        ptouch(tinyb)

        # Extraction: t1 = pb1 + bias carries a PE wait on bank1's stop
        # (o=63), which dominates bank0's stop (o=62) in PE wait history, so
        # the STT's PE component is elided and it carries only the t1 RAW.
        t1 = [
            const_pool.tile([O, CHUNK], mybir.dt.float32, name=f"t1_{ch}")
            for ch in range(NCH)
        ]
        for ch in range(NCH):
            nc.vector.tensor_scalar_add(
                t1[ch][:], pb[1][ch][0:O, :], b_sb[0:O, 0:1]
            )
        for ch in range(NCH):
            nc.vector.scalar_tensor_tensor(
                out_sb[0:O, CHUNK * ch : CHUNK * (ch + 1)],
                t1[ch][:],
                0.0,
                pb[0][ch][0:O, :],
                op0=mybir.AluOpType.add,
                op1=mybir.AluOpType.add,
            )

        nc.sync.dma_start(out_t[:, :], out_sb[:])

    return nc


def _prep_inputs(x, w, b):
    xs = x.reshape(B, N, C).astype(np.float32)
    wp = np.zeros((CP, O), dtype=np.float32)
    wp[:C, :] = w.astype(np.float32)
    bvec = b.astype(np.float32).reshape(O, 1)
    in_maps = []
    for core in range(N_CORES):
        xt = np.zeros((CP, N), dtype=ml_dtypes.bfloat16)
        xt[:C, :] = xs[core].T.astype(ml_dtypes.bfloat16)
        in_maps.append({"xt": xt, "wp": wp, "bvec": bvec})
    return in_maps


def kernel(x, w, b):
    from concourse.bass_utils import run_bass_kernel_spmd

    if "nc" not in _CACHE:
        _CACHE["nc"] = _build_program()
    nc = _CACHE["nc"]

    in_maps = _prep_inputs(x, w, b)
    res = run_bass_kernel_spmd(nc, in_maps, list(range(N_CORES)))
    out = np.stack(
        [np.asarray(res.results[core]["out_t"], dtype=np.float32).T for core in range(N_CORES)]
    )
    return out.astype(np.float32)



# revision 17
# speedup vs baseline: 3.8943x; 3.8943x over previous
import sys

for _p in ("/opt/trn_rl_repo", "/opt/trn_rl_repo/concourse"):
    if _p not in sys.path:
        sys.path.insert(0, _p)

import numpy as np
import ml_dtypes

N_CORES = 8
B, H, W_DIM, C = 8, 32, 32, 288
N = H * W_DIM          # 1024 points per core (batch-dim sharding: 1 image per core)
O = 64                 # codewords
SLAB = 32              # c-slab height; 288 = 9 slabs, zero padding
NSLAB = C // SLAB      # 9
GRP = 4                # o's packed per 128-partition tile (4 x 32)
NGRP = O // GRP        # 16
CHUNK = 512            # PSUM bank free size (fp32)
NCH = N // CHUNK       # 2 chunks
PTBUF = 18             # p-tile ring depth (2 full groups)
CP3 = 384              # C padded to 3 full 128-partition tiles (for -Sx)

_CACHE = {}
_DEBUG_NAMES = {}


def _patch_drain_split():
    # The end-of-TileContext drain waits on the FULL global clock (PE + DVE
    # + one sem per DMA HW queue), overflowing the CTRL_NO struct's
    # sync-wait slots in walrus. Split: emit one 1-wait SP nop per clock
    # component first; the original drain's full-clock add_sem_waits then
    # elides everything via SP wait history.
    import concourse.tile as tile_mod
    from concourse.vector_clock import ScopedClock, VectorClock

    if getattr(tile_mod.TileContext, "_drain_split_patched", False):
        return

    def _drain_and_barrier(self, tick_clock, wait_clock):
        gc = tick_clock.global_clock
        for idx in range(len(gc)):
            tick = gc[idx]
            if tick <= 0:
                continue
            nop = self.nc.sync.nop(nofuse=True, hint="drain_split")
            vc = VectorClock()
            vc.require_at_least(idx, tick)
            wait_clock.add_sem_waits(nop.ins, ScopedClock({None: vc}))
        # Waitless drain: the nops above (same SP sequencer, in order)
        # already guarantee every sem is at its final value here.
        self.nc.sync.drain()
        self.nc.all_engine_barrier()
        assert self.sems is not None
        popped = self.nc._tile_sem_poison_stack.pop()
        assert popped is self._sem_poison
        self.nc.clear_and_free_semaphores(list(self.sems.allocated().values()))
        self.nc.all_engine_barrier()

    tile_mod.TileContext._drain_and_barrier = _drain_and_barrier
    tile_mod.TileContext._drain_split_patched = True


def _build_program():
    import concourse.bass as bass
    import concourse.tile as tile
    from concourse import mybir

    _patch_drain_split()
    nc = bass.Bass("TRN2", debug=False, num_devices=N_CORES)

    # xrep: slab s (32 c's) replicated 4x across the partition dim, bf16.
    xrep_d = nc.dram_tensor("xrep", [NSLAB * 128, N], mybir.dt.bfloat16, kind="ExternalInput")
    # xt3: plain transposed x, zero-padded to 384 c's (for the -Sx matmuls).
    xt3_d = nc.dram_tensor("xt3", [CP3, N], mybir.dt.bfloat16, kind="ExternalInput")
    # wneg: column 9*g+s = -w packed per (group, slab): [32k+i] = -w[32s+i, 4g+k]
    wneg_d = nc.dram_tensor("wneg", [128, NGRP * NSLAB], mybir.dt.float32, kind="ExternalInput")
    # b2: [p, j] = (b[j] + sum_c w[c,j]) / 128, bf16 (bias via rank-1 matmul)
    b2_d = nc.dram_tensor("b2", [128, O], mybir.dt.bfloat16, kind="ExternalInput")
    out_d = nc.dram_tensor("out_t", [O, N], mybir.dt.float32, kind="ExternalOutput")

    xrep = xrep_d.ap()
    xt3 = xt3_d.ap()
    wneg = wneg_d.ap()
    b2 = b2_d.ap()
    out_t = out_d.ap()

    from contextlib import ExitStack

    from concourse.tile import add_dep_helper

    with tile.TileContext(nc) as tc, ExitStack() as ctx:
        const_pool = ctx.enter_context(tc.tile_pool(name="const", bufs=1))
        psum_pool = ctx.enter_context(tc.tile_pool(name="ps", bufs=1, space="PSUM"))

        # Walrus TensorScalar/Activation ISA structs fit ONE sync wait.
        # Every DMA gets a tiny DVE "touch" so later DVE consumers carry the
        # DMA-queue wait in DVE history; all DVE-sourced deps merge into the
        # single per-engine sem component.
        scratch = const_pool.tile([1, 128], mybir.dt.float32)
        touch_col = [0]

        def touch(src_ap):
            k = touch_col[0]
            touch_col[0] += 1
            bi = nc.vector.tensor_scalar_add(scratch[0:1, k : k + 1], src_ap, 0.0)
            return bi, k

        def touch_write(tile_obj):
            # write into a ring slot, reading only the long-quiet scratch
            # col 127 so the sole fresh wait is the slot's PE reader clock.
            # The write straddles the chunk boundary so it WARs against
            # BOTH chunk matmuls (subtile deps track per-range readers).
            bi = nc.vector.tensor_scalar_add(
                tile_obj[0:1, CHUNK - 1 : CHUNK + 1], scratch[0:1, 126:128], 0.0
            )
            return bi, None

        nc.vector.memset(scratch[:], 0.0)

        in_dmas = []

        wneg_sb = const_pool.tile([128, NGRP * NSLAB], mybir.dt.float32)
        in_dmas.append(nc.sync.dma_start(wneg_sb[:], wneg[:, :]))
        touch(wneg_sb[0:1, 0:1])

        x_sb = []
        for s in range(NSLAB):
            xs = const_pool.tile([128, N], mybir.dt.bfloat16, name=f"x_sb{s}")
            in_dmas.append(nc.sync.dma_start(xs[:], xrep[128 * s : 128 * (s + 1), :]))
            touch(xs[0:1, 0:1])
            x_sb.append(xs)

        b2_sb = const_pool.tile([128, O], mybir.dt.bfloat16)
        in_dmas.append(nc.sync.dma_start(b2_sb[:], b2[:, :]))
        touch(b2_sb[0:1, 0:1])

        xt3_sb = []
        for t in range(3):
            xs = const_pool.tile([128, N], mybir.dt.bfloat16, name=f"xt3_{t}")
            in_dmas.append(nc.sync.dma_start(xs[:], xt3[128 * t : 128 * (t + 1), :]))
            touch(xs[0:1, 0:1])
            xt3_sb.append(xs)

        # zwin[p, 64 + p//32] = 2.0 else 0. lhsT for group g = zwin[:, 64-4g :
        # 128-4g]: window column j holds the 2.0-block for output partition j
        # exactly when j = 4g + p//32 — routes 2*sum_c(relu) of o-block k
        # onto PSUM partition 4g+k.
        zwin = const_pool.tile([128, 128], mybir.dt.bfloat16)
        nc.vector.memset(zwin[:], 0.0)
        for k in range(GRP):
            nc.vector.memset(zwin[32 * k : 32 * (k + 1), 64 + k : 65 + k], 2.0)

        # all-(-1) lhsT: -Sx[n] accumulated onto every output partition
        neg1 = const_pool.tile([128, O], mybir.dt.bfloat16)
        nc.vector.memset(neg1[:], -1.0)
        # all-ones rhs for the rank-1 bias matmul
        ones = const_pool.tile([128, CHUNK], mybir.dt.bfloat16)
        nc.vector.memset(ones[:], 1.0)

        ps = [
            psum_pool.tile([O, CHUNK], mybir.dt.float32, name=f"ps{ch}")
            for ch in range(NCH)
        ]

        # p-tile ring: 18 fixed tiles (2 groups deep). TS(i) rewrites slot
        # i%18; the WAR against that slot's old PE readers and the WAW
        # against its old DVE writer are pre-absorbed into DVE wait history
        # once per group (rt/at/wt below) so each real TS carries <=1 wait.
        pt = [
            const_pool.tile([128, N], mybir.dt.bfloat16, name=f"pt{j}")
            for j in range(PTBUF)
        ]

        # out[o, n] = 2*sum_c relu(x-w) - Sx[n] + Sw[o] + b[o]
        for g in range(NGRP):
            wt = None
            if g >= 2:
                # slot of the LAST TS of group g-2: all of group g's ring
                # slots have older writers/readers, so absorbing this slot's
                # clocks covers the whole group.
                slot_t = (9 * g - 10) % PTBUF
                # rt: RAW on the slot's DVE writer -> loads the max old
                # DVE write value into history (1 wait).
                rt, krt = touch(pt[slot_t][0:1, 0:1])
                # at: RAW on rt's scratch col -> loads v(rt) itself into
                # history so wt's reader-WAR on rt is elided (1 wait).
                at, _ = touch(scratch[0:1, krt : krt + 1])
                # wt: write into the slot -> waits its PE readers (1 wait),
                # DVE components elided via rt/at.
                wt, _ = touch_write(pt[slot_t])
                _DEBUG_NAMES[rt.ins.name] = f"rt{g}"
                _DEBUG_NAMES[at.ins.name] = f"at{g}"
                _DEBUG_NAMES[wt.ins.name] = f"wt{g}"

            for s in range(NSLAB):
                i = NSLAB * g + s
                col = i
                p = pt[i % PTBUF]
                ts = nc.vector.tensor_scalar(
                    p[:], x_sb[s][:], wneg_sb[:, col : col + 1], 0.0,
                    op0=mybir.AluOpType.add,
                    op1=mybir.AluOpType.max,
                )
                _DEBUG_NAMES[ts.ins.name] = f"ts{i}"
                if wt is not None:
                    # scheduling-only edge: keep every TS of this group
                    # after the group's absorber, so the PE wait is always
                    # in DVE history when the TS is placed.
                    add_dep_helper(ts.ins, wt.ins, sync=False,
                                   reason="ts after group absorber")
                for ch in range(NCH):
                    nc.tensor.matmul(
                        ps[ch][:],
                        lhsT=zwin[:, 64 - 4 * g : 128 - 4 * g],
                        rhs=p[:, CHUNK * ch : CHUNK * (ch + 1)],
                        start=(g == 0 and s == 0),
                        stop=False,
                    )

        # -Sx[n]: sum over all c (3 padded 128-tiles) with weight -1
        for t in range(3):
            for ch in range(NCH):
                nc.tensor.matmul(
                    ps[ch][:],
                    lhsT=neg1[:, 0:O],
                    rhs=xt3_sb[t][:, CHUNK * ch : CHUNK * (ch + 1)],
                    start=False,
                    stop=False,
                )
        # + (b[o] + Sw[o]): rank-1 matmul, lhsT column j = (b[j]+Sw[j])/128
        for ch in range(NCH):
            nc.tensor.matmul(
                ps[ch][:],
                lhsT=b2_sb[:, 0:O],
                rhs=ones[:, :],
                start=False,
                stop=True,
            )

        out_sb = const_pool.tile([O, N], mybir.dt.float32)
        for ch in range(NCH):
            nc.vector.tensor_scalar_add(
                out_sb[:, CHUNK * ch : CHUNK * (ch + 1)], ps[ch][:], 0.0
            )
        # A HWDGE out-DMA descriptor fits one sync wait, but the hardware
        # queue also forces a ring-ordering wait behind the input DMAs --
        # two waits, which walrus can't encode. GpSimd issues DMAs in
        # software with no such limit.
        nc.gpsimd.dma_start(out_t[:, :], out_sb[:])

    return nc


def _prep_inputs(x, w, b):
    xs = x.reshape(B, N, C).astype(np.float32)
    wf = w.astype(np.float32)

    # wneg[32k+i, 9g+s] = -w[32s+i, 4g+k]
    wneg = np.empty((128, NGRP * NSLAB), dtype=np.float32)
    for g in range(NGRP):
        for s in range(NSLAB):
            blk = -wf[SLAB * s : SLAB * (s + 1), GRP * g : GRP * (g + 1)]  # [32, 4]
            wneg[:, NSLAB * g + s] = blk.T.reshape(128)

    sw = wf.sum(axis=0, dtype=np.float64)
    b2row = ((b.astype(np.float64) + sw) / 128.0).astype(ml_dtypes.bfloat16)
    b2 = np.broadcast_to(b2row, (128, O)).copy()

    in_maps = []
    for core in range(N_CORES):
        xt = xs[core].T.astype(ml_dtypes.bfloat16)  # [288, 1024]
        xrep = np.empty((NSLAB * 128, N), dtype=ml_dtypes.bfloat16)
        for s in range(NSLAB):
            slab = xt[SLAB * s : SLAB * (s + 1), :]  # [32, 1024]
            xrep[128 * s : 128 * (s + 1), :] = np.tile(slab, (GRP, 1))
        xt3 = np.zeros((CP3, N), dtype=ml_dtypes.bfloat16)
        xt3[:C, :] = xt
        in_maps.append({"xrep": xrep, "xt3": xt3, "wneg": wneg, "b2": b2})
    return in_maps


def kernel(x, w, b):
    from concourse.bass_utils import run_bass_kernel_spmd

    if "nc" not in _CACHE:
        _CACHE["nc"] = _build_program()
    nc = _CACHE["nc"]

    in_maps = _prep_inputs(x, w, b)
    res = run_bass_kernel_spmd(nc, in_maps, list(range(N_CORES)))
    out = np.stack(
        [np.asarray(res.results[core]["out_t"], dtype=np.float32).T for core in range(N_CORES)]
    )
    return out.astype(np.float32)
